# revision 32
# baseline (speedup 1.0000x reference)
"""BitLinear forward (fake-quant int8 activations x ternary weight) on 8 TRN2 cores.

Strategy (data-parallel over tokens, fp8 DoubleRow matmuls):
  - Shard x over the flattened (B*S) token dim: 8192 rows per core.
  - Host marshals x to a transposed, pre-scaled fp16 layout
    xt[p, b, s] = x[s, 128b+p] / scale so the contraction dim lands on SBUF
    partitions with no on-device transpose; fp16 keeps DMA at 512B
    descriptors when s-tiles are loaded in 256-column pairs. Host packs the
    ternary weight as fp8e4 wt[p, b, o] = w.T[128b+p, o] (exact: {-1,0,1})
    and replicates bias/scale per core.
  - Per 256-column pair of output tiles:
      Pool  u  = xt + 1.5*2^23        (magic round-to-nearest-even in fp32)
      ACT   hi = fp8(u - M)           (fp8e4 cast of the int8 value)
      ACT   q  = bf16(u - M)          (blocks 3..8 only, feeds Pool)
      DVE   lo[0:3] = (u - M) - hi    (exact residual, |lo| <= 4)
      Pool  lo[3:8] = q - hi
      PE    psum[s,o] += hi.T @ w + lo.T @ w as fp8 DoubleRow matmuls
            (both operands fp8e4, 2 k-tiles per instruction, 0.5 cyc/col:
            4x the bf16 MAC rate; hi+lo costs 2x -> net 2x vs bf16, exact
            since all products/sums are small integers in fp32 PSUM)
      DVE   out = psum*scale + bias -> fp16
      DMA out (SP ring).
  The quantize clamp to [-127,127] is dropped: act_scale = max|x|/127 by
  construction, so |round(x/scale)| <= 127 always.

Engine budget per 256-col pair (cost model): PE 32 DR matmuls @107 = 3413 ns
(the bottleneck, ~100% busy mid-stream), DVE ~3240, ACT ~3260, Pool ~2830,
DMA in+out 2912 ns. Fill/drain tuning: the first two pairs are processed in
2-block slices quantized on DVE (Pool generates const-DMA descriptors for
the first ~5 us), warmup matmuls pre-ramp the PE p-state, and the last pair
runs h-major with split drains. Cost model: 119.4 us per core vs 256.3 us
for the bf16 baseline (2.15x); pure-matmul floor is 109.3 us.
"""

import numpy as np
import ml_dtypes

B, S, D = 16, 4096, 1024
N_CORES = 8
ROWS = (B * S) // N_CORES  # 8192 rows per core
P = 128
KT = D // P                # 8 k-blocks
PAIR = 256                 # s-columns per input DMA (512B descriptors)
NPAIR = ROWS // PAIR       # 32 pairs per core
QB = 127.0
MAGIC = float(1.5 * 2 ** 23)

_NC_CACHE = {}


def _build_nc(npair=NPAIR, lo_dve_blocks=3, xin_bufs=4, u_bufs=3, q_bufs=3,
              out_bufs=4, po_bufs=4, fine_pairs=2, warmup_mms=9,
              tail_quarters=2, hoist2=False, pool_quant_hoisted=False):
    import concourse.mybir as mybir
    from concourse import bacc
    from concourse.tile import TileContext

    fp32 = mybir.dt.float32
    fp16 = mybir.dt.float16
    bf16 = mybir.dt.bfloat16
    fp8 = mybir.dt.float8e4
    Alu = mybir.AluOpType
    Act = mybir.ActivationFunctionType
    DR = mybir.MatmulPerfMode.DoubleRow

    nc = bacc.Bacc(None, target_bir_lowering=False)
    rows = npair * PAIR
    xt = nc.dram_tensor("xt", [P, KT, rows], fp16, kind="ExternalInput")
    wt = nc.dram_tensor("wt", [P, KT, D], fp8, kind="ExternalInput")
    bias_b = nc.dram_tensor("bias_b", [P, D], fp32, kind="ExternalInput")
    scal = nc.dram_tensor("scal", [P, 2], fp32, kind="ExternalInput")  # [scale, 1/scale]
    out = nc.dram_tensor("out", [rows, D], fp16, kind="ExternalOutput")

    bs = lo_dve_blocks
    NG = KT // 2  # 4 DoubleRow k-groups

    with TileContext(nc) as tc:
        with (
            tc.tile_pool(name="const", bufs=1) as constp,
            tc.tile_pool(name="xin", bufs=xin_bufs) as xp,
            tc.tile_pool(name="up", bufs=u_bufs) as up,
            tc.tile_pool(name="qp", bufs=q_bufs) as qp,
            tc.tile_pool(name="pop", bufs=po_bufs, space="PSUM") as pop,
            tc.tile_pool(name="oout", bufs=out_bufs) as op_,
        ):
            if warmup_mms:
                # dummy matmuls start the PE p-state ramp clock (~3 us to
                # full speed) while the first x tiles and quantize passes
                # are still in flight, so the real matmuls run at 2.4 GHz
                # almost immediately. Sized so the warmup stream ends just
                # before the first real matmul is ready: the ramp clock
                # resets if the PE sits idle for long.
                w0 = constp.tile([P, 2, 512], fp8)
                nc.gpsimd.memset(w0, 0)
                pw = pop.tile([P, 512], fp32, name="po")
                for _ in range(warmup_mms):
                    nc.tensor.matmul(pw, w0[:, :, 0:128], w0,
                                     start=True, stop=True,
                                     perf_mode=DR)

            # const DMAs ride the gpsimd SWDGE: they occupy the Pool engine
            # for ~5 us, so the fine fill pairs below run their quantize on
            # DVE instead of Pool
            sc = constp.tile([P, 2], fp32)
            nc.gpsimd.dma_start(out=sc, in_=scal[:, :])
            wt_sb = constp.tile([P, KT, D], fp8)
            # per-k-group chunks so the first matmuls only wait for their
            # own weights while the first x slices stream in
            for g in range(NG):
                nc.gpsimd.dma_start(out=wt_sb[:, 2 * g:2 * g + 2, :],
                                    in_=wt[:, 2 * g:2 * g + 2, :])
            bias_sb = constp.tile([P, D], fp32)
            # bias is first needed at the first PSUM drain (~7 us in)
            nc.gpsimd.dma_start(out=bias_sb, in_=bias_b[:, :])

            def mm(po, pdat, g, sl, h, start, stop):
                nc.tensor.matmul(
                    po[:, h * 512:(h + 1) * 512],
                    pdat[:, 2 * g:2 * g + 2, sl],
                    wt_sb[:, 2 * g:2 * g + 2, h * 512:(h + 1) * 512],
                    start=start, stop=stop, perf_mode=DR,
                )

            # pair 2's input DMA is hoisted ahead of pair 1's sliced loads so
            # its data is on-chip by the time Pool finishes const-DMA
            # descriptor generation; pair 2 then quantizes entirely on Pool
            # (DVE/ACT are saturated by the fine pairs + first drains)
            hoisted = {}
            order = []
            for pr in range(npair):
                if hoist2 and pr == 1 and fine_pairs == 2 and npair > 3:
                    order.append(("dma2", 2))
                order.append(("pair", pr))

            for kind, pr in order:
                s0 = pr * PAIR
                if kind == "dma2":
                    xa2 = xp.tile([P, KT, PAIR], fp16, name="xa")
                    nc.sync.dma_start(out=xa2, in_=xt[:, :, s0:s0 + PAIR])
                    hoisted[pr] = xa2
                    continue
                fine = pr < fine_pairs
                is_last = pr == npair - 1
                pool_quant = (pr in hoisted) and pool_quant_hoisted

                xa = hoisted.get(pr)
                if xa is None:
                    xa = xp.tile([P, KT, PAIR], fp16, name="xa")
                u = up.tile([P, KT, PAIR], fp32, name="u")
                hi = qp.tile([P, KT, PAIR], fp8, name="hi")
                lo = qp.tile([P, KT, PAIR], fp8, name="lo")

                if fine:
                    # fill the pipeline at k-group granularity: DMA, quant,
                    # hi and lo per slice. All quantize work on DVE/ACT:
                    # Pool is generating const-DMA descriptors. lo slices
                    # lag one slice behind u slices on the DVE queue so the
                    # hi-matmul chain isn't serialized on lo. Pair 0 uses
                    # 1-group slices for the fastest start, later fine pairs
                    # 2-group slices for lower instruction overhead.
                    gper = 1
                    nsl = NG // gper
                    # pairs 0-1 quantize on DVE (Pool is busy with const-DMA
                    # descriptor generation); later fine pairs go back to
                    # Pool, which frees up just in time
                    u_eng = nc.vector if pr < 2 else nc.gpsimd

                    def fine_u(i):
                        gs = slice(2 * gper * i, 2 * gper * (i + 1))
                        nc.sync.dma_start(out=xa[:, gs, :],
                                          in_=xt[:, gs, s0:s0 + PAIR])
                        u_eng.tensor_scalar(u[:, gs, :], xa[:, gs, :],
                                            MAGIC, None, Alu.add)
                        nc.scalar.activation(hi[:, gs, :], u[:, gs, :],
                                             Act.Copy, bias=-MAGIC)

                    def fine_lo(i):
                        gs = slice(2 * gper * i, 2 * gper * (i + 1))
                        nc.vector.scalar_tensor_tensor(
                            lo[:, gs, :], u[:, gs, :], MAGIC, hi[:, gs, :],
                            Alu.subtract, Alu.subtract)

                    fine_u(0)
                    for i in range(1, nsl):
                        fine_u(i)
                        fine_lo(i - 1)
                    fine_lo(nsl - 1)
                else:
                    if pr not in hoisted:
                        nc.sync.dma_start(out=xa, in_=xt[:, :, s0:s0 + PAIR])

                    # u = x/scale + M (fp32; M forces round-to-nearest-even
                    # of the int8 value into the low mantissa bits)
                    nc.gpsimd.tensor_scalar(u, xa, MAGIC, None, Alu.add)

                    # hi = fp8(u - M): the fp8-rounded int8 value
                    if pool_quant:
                        nc.gpsimd.tensor_scalar(hi, u, MAGIC, None,
                                                Alu.subtract)
                    else:
                        nc.scalar.activation(hi, u, Act.Copy, bias=-MAGIC)

                    # lo = (u - M) - hi: exact fp8 residual. DVE handles the
                    # first blocks via stt; Pool (no stt opcode) gets a bf16
                    # q from ACT and subtracts with tensor_tensor.
                    if bs > 0:
                        nc.vector.scalar_tensor_tensor(
                            lo[:, 0:bs, :], u[:, 0:bs, :], MAGIC,
                            hi[:, 0:bs, :], Alu.subtract, Alu.subtract)
                    if bs < KT:
                        q = qp.tile([P, KT - bs, PAIR], bf16, name="q")
                        nc.scalar.activation(q, u[:, bs:KT, :], Act.Copy,
                                             bias=-MAGIC)
                        nc.gpsimd.tensor_tensor(
                            lo[:, bs:KT, :], q, hi[:, bs:KT, :], Alu.subtract)

                for tp in range(2):
                    sl = slice(tp * P, (tp + 1) * P)
                    po = pop.tile([P, D], fp32, name="po")
                    oo = op_.tile([P, D], fp16, name="oo")
                    if fine:
                        # g-major so each slice's matmuls issue as soon as
                        # its hi/lo land
                        for g in range(NG):
                            for part, pdat in ((0, hi), (1, lo)):
                                for h in range(2):
                                    mm(po, pdat, g, sl, h,
                                       start=(part == 0 and g == 0),
                                       stop=(part == 1 and g == NG - 1))
                    elif is_last:
                        # n-major: finish one psum n-chunk completely, drain
                        # it and ship it while the next chunk's matmuls run;
                        # the tail after the very last matmul is one small
                        # drain + one small DMA. tp1 (the true tail) uses
                        # quarter chunks, tp0 halves.
                        nq = tail_quarters if tp == 1 else 2
                        w = D // nq
                        for qi in range(nq):
                            qs = slice(qi * w, (qi + 1) * w)
                            for part, pdat in ((0, hi), (1, lo)):
                                for g in range(NG):
                                    nc.tensor.matmul(
                                        po[:, qs],
                                        pdat[:, 2 * g:2 * g + 2, sl],
                                        wt_sb[:, 2 * g:2 * g + 2, qs],
                                        start=(part == 0 and g == 0),
                                        stop=(part == 1 and g == NG - 1),
                                        perf_mode=DR)
                            nc.vector.scalar_tensor_tensor(
                                oo[:, qs], po[:, qs], sc[:, 0:1],
                                bias_sb[:, qs], Alu.mult, Alu.add)
                            eng = nc.scalar if (tp + qi) % 2 else nc.sync
                            eng.dma_start(
                                out=out[s0 + tp * P:s0 + (tp + 1) * P, qs],
                                in_=oo[:, qs])
                        continue
                    else:
                        for part, pdat in ((0, hi), (1, lo)):
                            for g in range(NG):
                                for h in range(2):
                                    mm(po, pdat, g, sl, h,
                                       start=(part == 0 and g == 0),
                                       stop=(part == 1 and g == NG - 1))
                    # out = psum * scale + bias -> fp16 (DVE; gpsimd cannot
                    # read PSUM)
                    nc.vector.scalar_tensor_tensor(
                        oo, po, sc[:, 0:1], bias_sb, Alu.mult, Alu.add)
                    nc.sync.dma_start(
                        out=out[s0 + tp * P:s0 + (tp + 1) * P, :], in_=oo)
    nc.compile()
    return nc


def _get_nc():
    if "nc" not in _NC_CACHE:
        _NC_CACHE["nc"] = _build_nc()
    return _NC_CACHE["nc"]


def _prep_inputs(x, ternary_weight, bias, act_scale, n_cores=N_CORES, rows=ROWS):
    x = np.asarray(x, dtype=np.float32)
    tw = np.asarray(ternary_weight)
    bias = np.asarray(bias, dtype=np.float32)

    scale = np.maximum(np.float32(act_scale), np.float32(1e-5))
    inv = np.float32(1.0) / scale

    # wt[p, b, o] = tw[o, 128b+p] - 1, exact in fp8e4
    wtT = tw.T.astype(np.float32) - 1.0  # [D_IN, D_OUT]
    wt8 = np.ascontiguousarray(
        wtT.reshape(KT, P, D).transpose(1, 0, 2)
    ).astype(ml_dtypes.float8_e4m3)
    bias_b = np.ascontiguousarray(np.broadcast_to(bias[None, :], (P, D)))
    scal = np.ascontiguousarray(
        np.broadcast_to(np.array([scale, inv], dtype=np.float32)[None, :], (P, 2))
    )

    # xt[p, b, s] = x[s, 128b+p] / scale in fp16 (one big transpose +
    # scale + cast, then per-core repack)
    xf = x.reshape(-1, D)
    xt_all = (xf.T * inv).astype(np.float16)  # [D, B*S]
    in_maps = []
    for c in range(n_cores):
        xc = xt_all[:, c * rows:(c + 1) * rows]          # [1024, rows] view
        xt_c = np.ascontiguousarray(
            xc.reshape(KT, P, rows).transpose(1, 0, 2))  # [128, 8, rows]
        in_maps.append({
            "xt": xt_c,
            "wt": wt8,
            "bias_b": bias_b,
            "scal": scal,
        })
    return in_maps


def kernel(x, ternary_weight, bias, act_scale):
    from concourse.bass_utils import run_bass_kernel_spmd

    in_maps = _prep_inputs(x, ternary_weight, bias, act_scale)
    nc = _get_nc()
    res = run_bass_kernel_spmd(nc, in_maps, core_ids=list(range(N_CORES)))
    out = np.concatenate(
        [np.asarray(r["out"]).astype(np.float32) for r in res.results], axis=0)
    return out.reshape(B, S, D)
